# revision 18
# baseline (speedup 1.0000x reference)
"""CLIP-style loss kernel for Trainium2 (8 NeuronCores, SPMD data-parallel).

Problem: two patch-embeddings (stride-4 4x4 conv -> L2 normalize) of
imgs/hha [32,64,128,128], per-sample logits = exp(logit_scale) * a @ h^T
[B,1024,1024], symmetric cross-entropy with diagonal labels, scalar loss.

Sharding: data-parallel over batch, 4 samples per core. Host does the
im2col permutation + fp8 cast (free: stride==kernel so im2col is a pure
permutation); each core computes, per sample:
  conv:    fp8 DoubleRow matmuls (K=256 per step, 2x bf16 rate):
           Y[d,n] accumulated over 4 k-pair chunks per 512-patch half.
  norms:   sq = (Y+b)^2 in bf16; norm2 rows via ones-stationary matmuls
           into one PSUM bank at partitions {0,32,64,96}; one Ln + one
           Exp(-0.5*) ACT op yields inverse norms for both modalities.
  hats:    K=1 ones-row matmul broadcasts inv rows to [128, n] (logit
           scale folded into the a-side broadcast stationary); DVE muls
           give a_hat (= s * Ya * invA) and h_hat in bf16.
  logits:  per 128-row chunk: 2 bf16 matmuls -> L [128,1024] PSUM;
           one ACT Exp -> E bf16; DVE row-sum -> RS; ones-stationary
           matmuls accumulate colsum in PSUM across chunks.
  diag:    t = a_hat .* h_hat; ones-stationary matmuls -> diag row.
Outputs per core: row-sums RS [128, 4*8] and (colsum|diag) rows
[4 partitions, 4*512]; host finishes with log/sums in float64.
"""

import math
import os
import sys
from contextlib import ExitStack

import numpy as np

for _p in ("/opt/trn_rl_repo", "/root/.axon_site/_ro/trn_rl_repo"):
    if os.path.isdir(_p) and _p not in sys.path:
        sys.path.insert(0, _p)

import concourse.bass as bass
import concourse.mybir as mybir
import concourse.tile as tile
from concourse import bacc
from concourse.bass_utils import run_bass_kernel_spmd

F32 = mybir.dt.float32
BF16 = mybir.dt.bfloat16
FP8 = mybir.dt.float8e4
AF = mybir.ActivationFunctionType
ALU = mybir.AluOpType
DR = mybir.MatmulPerfMode.DoubleRow

N_CORES = 8
B_FULL = 32
BPC = B_FULL // N_CORES  # samples per core
C, H, W, D, P = 64, 128, 128, 128, 4
NPAT = (H // P) * (W // P)  # 1024 patches
NH = NPAT // 2  # 512
NOFF = P * P  # 16 kernel offsets
NCHUNK = NPAT // 128  # 8 logit row chunks
NKC = (C * NOFF) // 128  # 8 conv contraction chunks (K=128 each)

USE_DR = os.environ.get("BASS_NO_DR", "") != "1"


NLE = "natural_log_exp_and_others"


class _BaccPinned(bacc.Bacc):
    """Bacc whose act-table pass resolves Ln/Exp only to the combined
    'natural_log_exp_and_others' set, so a single ACT_TABLE_LOAD serves the
    whole program (stock set assignment alternates two sets, ~1.4us/swap)."""

    def insert_act_table_loads(self):
        import bass_rust as _bass_rust
        from concourse.hw_specs import get_activation_tables

        has_activation = any(
            isinstance(i, mybir.InstActivation)
            for b in self.main_func.blocks
            for i in b.instructions
        )
        if not has_activation:
            return
        pinned = {AF.Ln, AF.Exp}
        tables = [
            (name, funcs if name == NLE else (funcs - pinned))
            for name, funcs in get_activation_tables(self.m.arch).items()
        ]
        _bass_rust.insert_act_table_loads(self, tables)


def build_program(ln_s: float) -> bass.Bass:
    os.environ.pop("BASS_ACT_ROOT_JSON_PATH", None)
    s_val = float(math.exp(ln_s))
    nc = _BaccPinned(None)
    # host-im2col'd images: [b, k%128, k//128, patch] fp8
    imgs = nc.declare_dram_parameter("imgs", [BPC, 128, NKC, NPAT], FP8, isOutput=False)
    hha = nc.declare_dram_parameter("hha", [BPC, 128, NKC, NPAT], FP8, isOutput=False)
    w1t = nc.declare_dram_parameter("w1t", [128, NKC, D], FP8, isOutput=False)
    w2t = nc.declare_dram_parameter("w2t", [128, NKC, D], FP8, isOutput=False)
    b1 = nc.declare_dram_parameter("b1", [D], F32, isOutput=False)
    b2 = nc.declare_dram_parameter("b2", [D], F32, isOutput=False)
    # row sums of E per (sample, chunk): host does sum(log(.))
    out_rs = nc.declare_dram_parameter("out_rs", [128, BPC * NCHUNK], F32, isOutput=True)
    # rows {0,32,64,96} x [BPC*512]: colsum half0, colsum half1, diag half0, diag half1
    out_rows = nc.declare_dram_parameter("out_rows", [4, BPC * NH], F32, isOutput=True)
    dbg = os.environ.get("BASS_DEBUG_DUMP", "")
    out_dbg = (
        nc.declare_dram_parameter("out_dbg", [128, NPAT], F32, isOutput=True)
        if dbg
        else None
    )

    srcs = (imgs, hha)

    with tile.TileContext(nc) as tc, ExitStack() as ctx:
        p_one = ctx.enter_context(tc.tile_pool(name="singles", bufs=1))
        p_img = ctx.enter_context(tc.tile_pool(name="img", bufs=4))
        p_ym = ctx.enter_context(tc.tile_pool(name="ym", bufs=3))
        p_sq = ctx.enter_context(tc.tile_pool(name="sq", bufs=3))
        p_hat = ctx.enter_context(tc.tile_pool(name="hat", bufs=4))
        p_inv = ctx.enter_context(tc.tile_pool(name="inv", bufs=2))
        p_E = ctx.enter_context(tc.tile_pool(name="E", bufs=3))
        p_t = ctx.enter_context(tc.tile_pool(name="t", bufs=2))
        # PSUM: conv/bc 2 banks + logits 2x2 banks + reduce-rows 2 banks = 8
        pp_cv = ctx.enter_context(tc.tile_pool(name="ppcv", bufs=1, space="PSUM"))
        pp_L = ctx.enter_context(tc.tile_pool(name="ppL", bufs=2, space="PSUM"))
        pp_R = ctx.enter_context(tc.tile_pool(name="ppR", bufs=2, space="PSUM"))

        # constants / weights
        ones_k = p_one.tile([128, 1], BF16)
        nc.vector.memset(ones_k, 1.0)
        bc_st = p_one.tile([128, 128], BF16)  # K=1 broadcast stationaries
        nc.vector.memset(bc_st, 1.0)
        nc.vector.memset(bc_st[0:1, :], s_val)  # a-side rows carry exp(logit_scale)
        nc.vector.memset(bc_st[32:33, :], s_val)
        wts = []
        biases = []
        for wsrc, bsrc in ((w1t, b1), (w2t, b2)):
            wt = p_one.tile([128, NKC, D], FP8, tag=f"wt_{wsrc.name}")
            nc.sync.dma_start(out=wt, in_=wsrc[:])
            wts.append(wt)
            bt = p_one.tile([128, 1], F32, tag=f"bias_{bsrc.name}")
            nc.sync.dma_start(out=bt, in_=bsrc[:].rearrange("(d one) -> d one", one=1))
            biases.append(bt)
        OUT_RS = p_one.tile([128, BPC * NCHUNK], F32)
        OUT2 = p_one.tile([128, BPC * NH], F32)

        # warmup matmuls absorb the weight-DMA waits into PE program order
        R_init = pp_R.tile([128, NH], F32, tag="R", name="R_init")
        for m in range(2):
            nc.tensor.matmul(
                R_init[:, m : m + 1], wts[m][:, 0, :], wts[m][:, 0, 0:1],
                start=True, stop=True,
            )

        st = {}  # per-sample state

        def absorber(out_tile, col, rhs, name):
            """Tiny matmul so the next real matmul keeps a single sync wait.
            Writes [0:1, col] of a region later cleared by a start=True mm."""
            nc.tensor.matmul(
                out_tile[0:1, col : col + 1], ones_k, rhs, start=True, stop=True,
            )

        def produce1(b):
            """conv for sample b (images DMA'd one iteration earlier)."""
            img_a, img_h = st[b]["img"]
            ys = {}
            for m in range(2):
                img = img_a if m == 0 else img_h
                conv = pp_cv.tile([128, NPAT], F32, tag="cv", name=f"conv_{b}_{m}")
                if m == 0 and b > 0:
                    # WAR vs DVE hat-muls(b-1) (bc borrowed this slot)
                    absorber(conv, 0, st[b - 1]["h_hat"][:, 0:1], f"abA_{b}")
                elif m == 1:
                    # WAR vs DVE bias/sq_a(b)
                    absorber(conv, 0, ys[0]["sq"][:, 0:1], f"abB_{b}")
                if USE_DR:
                    for t in range(2):
                        for kp in range(NKC // 2):
                            nc.tensor.matmul(
                                conv[:, t * NH : (t + 1) * NH],
                                wts[m][:, 2 * kp : 2 * kp + 2, :],
                                img[:, 2 * kp : 2 * kp + 2, t * NH : (t + 1) * NH],
                                start=(kp == 0), stop=(kp == NKC // 2 - 1),
                                perf_mode=DR,
                            )
                else:
                    for t in range(2):
                        for kc in range(NKC):
                            nc.tensor.matmul(
                                conv[:, t * NH : (t + 1) * NH],
                                wts[m][:, kc, :],
                                img[:, kc, t * NH : (t + 1) * NH],
                                start=(kc == 0), stop=(kc == NKC - 1),
                            )
                ym = p_ym.tile([128, NPAT], BF16, tag="ym", name=f"ym_{b}_{m}")
                nc.vector.tensor_scalar_add(ym, conv, biases[m])
                sq = p_sq.tile([128, NPAT], BF16, tag="sq", name=f"sq_{b}_{m}")
                nc.vector.tensor_mul(sq, ym, ym)
                ys[m] = {"ym": ym, "sq": sq}
                if b == 0 and m == 0 and dbg == "ym":
                    dbg_dump(ym)
            st[b]["ys"] = ys

        def produce2(b):
            """norm2 rows, inverse norms, broadcasts, hats for sample b."""
            ys = st[b]["ys"]
            R1 = pp_R.tile([128, NH], F32, tag="R", name=f"R1_{b}")
            if b > 0:
                # WAR vs ACT Ln(b-1) on this slot; invs(b-1) is ACT-produced
                absorber(R1, 0, st[b - 1]["invs"][:, 0:1], f"abE_{b}")
            # norm2 rows: (m, t) -> partition 32*(2m+t)
            for m in range(2):
                for t in range(2):
                    q = 32 * (2 * m + t)
                    nc.tensor.matmul(
                        R1[q : q + 1, :], ones_k,
                        ys[m]["sq"][:, t * NH : (t + 1) * NH],
                        start=True, stop=True, tile_position=(0, q),
                    )
            u = p_inv.tile([128, NH], F32, tag="u", name=f"u_{b}")
            nc.scalar.activation(out=u[0:97, :], in_=R1[0:97, :], func=AF.Ln)
            invs = p_inv.tile([128, NH], BF16, tag="invs", name=f"invs_{b}")
            nc.scalar.activation(out=invs[0:97, :], in_=u[0:97, :], func=AF.Exp, scale=-0.5)
            st[b]["invs"] = invs
            if b == 0 and dbg == "invs":
                dbg_dump2 = p_one.tile([128, NH], F32, tag="dbg2")
                nc.vector.tensor_copy(dbg_dump2, invs)
                nc.vector.tensor_copy(DBG[:, 0:NH], dbg_dump2)
            if b == 0 and dbg == "R1":
                nc.vector.tensor_copy(DBG[0:97, 0:NH], R1[0:97, :])
            # broadcasts into the conv slot (free until conv(b+1))
            hats = []
            for m in range(2):
                bc = pp_cv.tile([128, NPAT], F32, tag="cv", name=f"bc_{b}_{m}")
                if m == 0:
                    # WAR vs DVE bias_h(b): ym_h is bias_h's output
                    absorber(bc, 0, ys[1]["ym"][:, 0:1], f"abC_{b}")
                for t in range(2):
                    q = 32 * (2 * m + t)
                    nc.tensor.matmul(
                        bc[:, t * NH : (t + 1) * NH],
                        bc_st[q : q + 1, :], invs[q : q + 1, :],
                        start=True, stop=True, tile_position=(q, 0),
                    )
                hat = p_hat.tile([128, NPAT], BF16, tag="hat", name=f"hat_{b}_{m}")
                nc.vector.tensor_mul(hat, ys[m]["ym"], bc)
                hats.append(hat)
                if b == 0 and m == 0 and dbg == "hat":
                    dbg_dump(hat)
                if b == 0 and m == 0 and dbg == "bc":
                    dbg_dump(bc)
            st[b]["a_hat"], st[b]["h_hat"] = hats

        def consume(b):
            """logits + exp + row/col sums + diag for sample b."""
            a_hat, h_hat = st[b]["a_hat"], st[b]["h_hat"]
            R2 = pp_R.tile([128, NH], F32, tag="R", name=f"R2_{b}")
            absorber(R2, 0, h_hat[:, 0:1], f"abD_{b}")
            for k in range(NCHUNK):
                L = pp_L.tile([128, NPAT], F32, tag="L", name=f"L_{b}_{k}")
                for t in range(2):
                    nc.tensor.matmul(
                        L[:, t * NH : (t + 1) * NH],
                        a_hat[:, 128 * k : 128 * (k + 1)],
                        h_hat[:, t * NH : (t + 1) * NH],
                        start=True, stop=True,
                    )
                E = p_E.tile([128, NPAT], BF16, tag="E", name=f"E_{b}_{k}")
                nc.scalar.activation(
                    out=E, in_=L, func=AF.Exp,
                    accum_out=OUT_RS[:, NCHUNK * b + k : NCHUNK * b + k + 1],
                )
                for t in range(2):
                    nc.tensor.matmul(
                        R2[32 * t : 32 * t + 1, :], ones_k,
                        E[:, t * NH : (t + 1) * NH],
                        start=(k == 0), stop=(k == NCHUNK - 1),
                        tile_position=(0, 32 * t),
                    )
            t_ = p_t.tile([128, NPAT], BF16, tag="t", name=f"t_{b}")
            nc.vector.tensor_mul(t_, a_hat, h_hat)
            for t in range(2):
                nc.tensor.matmul(
                    R2[64 + 32 * t : 64 + 32 * t + 1, :], ones_k,
                    t_[:, t * NH : (t + 1) * NH],
                    start=True, stop=True, tile_position=(0, 64 + 32 * t),
                )
            nc.vector.tensor_copy(OUT2[0:97, b * NH : (b + 1) * NH], R2[0:97, :])

        def dma_imgs(b):
            img_a = p_img.tile([128, NKC, NPAT], FP8, tag="img", name=f"ia_{b}")
            nc.sync.dma_start(out=img_a, in_=srcs[0][b])
            img_h = p_img.tile([128, NKC, NPAT], FP8, tag="img", name=f"ih_{b}")
            nc.sync.dma_start(out=img_h, in_=srcs[1][b])
            st[b] = {"img": (img_a, img_h)}

        DBG = p_one.tile([128, NPAT], F32, name="DBG") if dbg else None

        def dbg_dump(src):
            if DBG is not None:
                nc.vector.tensor_copy(DBG, src)

        dma_imgs(0)
        dma_imgs(1)
        produce1(0)
        for b in range(BPC):
            produce2(b)
            if b + 1 < BPC:
                if b + 2 < BPC:
                    dma_imgs(b + 2)
                produce1(b + 1)
            consume(b)
        nc.sync.dma_start(out=out_rs[:], in_=OUT_RS)
        nc.sync.dma_start(out=out_rows[:], in_=OUT2[0:128:32, :])
        if DBG is not None:
            nc.sync.dma_start(out=out_dbg[:], in_=DBG)

    nc.compile()
    return nc


_PROGRAM_CACHE: dict = {}


def _get_program(ln_s: float) -> bass.Bass:
    key = round(float(ln_s), 9)
    if key not in _PROGRAM_CACHE:
        _PROGRAM_CACHE[key] = build_program(float(ln_s))
    return _PROGRAM_CACHE[key]


def make_in_maps(imgs, hha, w1, b1, w2, b2):
    """Shard full inputs into per-core input maps (host-side, cheap)."""
    import ml_dtypes

    fp8 = ml_dtypes.float8_e4m3

    def prep_w(w):
        # [D,C,P,P] -> [(c,di,dj)=1024, D] -> [k%128, k//128, D]
        wf = np.transpose(np.asarray(w), (1, 2, 3, 0)).reshape(C * NOFF, D)
        return np.ascontiguousarray(
            wf.reshape(NKC, 128, D).transpose(1, 0, 2)
        ).astype(fp8)

    def prep_x(x):
        # stride==kernel -> im2col is a permutation:
        # [B,C,H,W] -> [B, (c,di,dj)=1024, (i,j)=1024] -> [B,128,NKC,NPAT]
        B = x.shape[0]
        xp = np.asarray(x).reshape(B, C, H // P, P, W // P, P)
        xp = xp.transpose(0, 1, 3, 5, 2, 4).reshape(B, C * NOFF, NPAT)
        return np.ascontiguousarray(
            xp.reshape(B, NKC, 128, NPAT).transpose(0, 2, 1, 3)
        ).astype(fp8)

    w1t = prep_w(w1)
    w2t = prep_w(w2)
    imgs = prep_x(imgs)
    hha = prep_x(hha)
    b1 = np.ascontiguousarray(np.asarray(b1), dtype=np.float32)
    b2 = np.ascontiguousarray(np.asarray(b2), dtype=np.float32)
    maps = []
    for i in range(N_CORES):
        maps.append(
            {
                "imgs": np.ascontiguousarray(imgs[i * BPC : (i + 1) * BPC]),
                "hha": np.ascontiguousarray(hha[i * BPC : (i + 1) * BPC]),
                "w1t": w1t,
                "w2t": w2t,
                "b1": b1,
                "b2": b2,
            }
        )
    return maps


def combine_outputs(outs) -> np.float32:
    """Reduce per-core {out_rs, out_rows} partials to the scalar loss."""
    tot = np.float64(0.0)
    for o in outs:
        rs = np.asarray(o["out_rs"], dtype=np.float64)  # [128, BPC*8]
        rows = np.asarray(o["out_rows"], dtype=np.float64)  # [4, BPC*512]
        lse_row = np.log(rs).sum()
        colsum = np.concatenate([rows[0], rows[1]])  # [BPC*512]*2 -> all cols
        lse_col = np.log(colsum).sum()
        diag = rows[2].sum() + rows[3].sum()
        tot += 0.5 * (lse_row + lse_col) - diag
    return np.float32(tot / (B_FULL * NPAT))


def run_spmd(imgs, hha, w1, b1, w2, b2, logit_scale, **kwargs):
    """Run on the 8 cores; returns (loss, BassKernelResults)."""
    ln_s = float(np.asarray(logit_scale))
    nc = _get_program(ln_s)
    in_maps = make_in_maps(imgs, hha, w1, b1, w2, b2)
    res = run_bass_kernel_spmd(nc, in_maps, list(range(N_CORES)), **kwargs)
    return combine_outputs(res.results), res


def kernel(imgs, hha, w1, b1, w2, b2, logit_scale):
    loss, _ = run_spmd(imgs, hha, w1, b1, w2, b2, logit_scale)
    return loss


if __name__ == "__main__":
    # smoke test against a tiny numpy reference of the math
    rng = np.random.default_rng(0)
    imgs = rng.standard_normal((B_FULL, C, H, W), dtype=np.float32)
    hha = rng.standard_normal((B_FULL, C, H, W), dtype=np.float32)
    w1 = rng.standard_normal((D, C, P, P), dtype=np.float32) * 0.03
    w2 = rng.standard_normal((D, C, P, P), dtype=np.float32) * 0.03
    b1 = np.zeros(D, np.float32)
    b2 = np.zeros(D, np.float32)
    ls = np.float32(np.log(1.0 / 0.07))
    print(kernel(imgs, hha, w1, b1, w2, b2, ls))


# revision 19
# speedup vs baseline: 1.1555x; 1.1555x over previous
"""CLIP-style loss kernel for Trainium2 (8 NeuronCores, SPMD data-parallel).

Problem: two patch-embeddings (stride-4 4x4 conv -> L2 normalize) of
imgs/hha [32,64,128,128], per-sample logits = exp(logit_scale) * a @ h^T
[B,1024,1024], symmetric cross-entropy with diagonal labels, scalar loss.

Sharding: data-parallel over batch, 4 samples per core. Host does the
im2col permutation + fp8 cast (free: stride==kernel so im2col is a pure
permutation); each core computes, per sample:
  conv:    fp8 DoubleRow matmuls (K=256 per step, 2x bf16 rate):
           Y[d,n] accumulated over 4 k-pair chunks per 512-patch half.
  norms:   sq = (Y+b)^2 in bf16; norm2 rows via ones-stationary matmuls
           into one PSUM bank at partitions {0,32,64,96}; one Ln + one
           Exp(-0.5*) ACT op yields inverse norms for both modalities.
  hats:    K=1 ones-row matmul broadcasts inv rows to [128, n] (logit
           scale folded into the a-side broadcast stationary); DVE muls
           give a_hat (= s * Ya * invA) and h_hat in bf16.
  logits:  per 128-row chunk: 2 bf16 matmuls -> L [128,1024] PSUM;
           one ACT Exp -> E bf16; DVE row-sum -> RS; ones-stationary
           matmuls accumulate colsum in PSUM across chunks.
  diag:    t = a_hat .* h_hat; ones-stationary matmuls -> diag row.
Outputs per core: row-sums RS [128, 4*8] and (colsum|diag) rows
[4 partitions, 4*512]; host finishes with log/sums in float64.
"""

import math
import os
import sys
from contextlib import ExitStack

import numpy as np

for _p in ("/opt/trn_rl_repo", "/root/.axon_site/_ro/trn_rl_repo"):
    if os.path.isdir(_p) and _p not in sys.path:
        sys.path.insert(0, _p)

import concourse.bass as bass
import concourse.mybir as mybir
import concourse.tile as tile
from concourse import bacc
from concourse.bass_utils import run_bass_kernel_spmd

F32 = mybir.dt.float32
BF16 = mybir.dt.bfloat16
FP8 = mybir.dt.float8e4
AF = mybir.ActivationFunctionType
ALU = mybir.AluOpType
DR = mybir.MatmulPerfMode.DoubleRow

N_CORES = 8
B_FULL = 32
BPC = B_FULL // N_CORES  # samples per core
C, H, W, D, P = 64, 128, 128, 128, 4
NPAT = (H // P) * (W // P)  # 1024 patches
NH = NPAT // 2  # 512
NOFF = P * P  # 16 kernel offsets
NCHUNK = NPAT // 128  # 8 logit row chunks
NKC = (C * NOFF) // 128  # 8 conv contraction chunks (K=128 each)

USE_DR = os.environ.get("BASS_NO_DR", "") != "1"


NLE = "natural_log_exp_and_others"


class _BaccPinned(bacc.Bacc):
    """Bacc whose act-table pass resolves Ln/Exp only to the combined
    'natural_log_exp_and_others' set, so a single ACT_TABLE_LOAD serves the
    whole program (stock set assignment alternates two sets, ~1.4us/swap)."""

    def insert_act_table_loads(self):
        import bass_rust as _bass_rust
        from concourse.hw_specs import get_activation_tables

        has_activation = any(
            isinstance(i, mybir.InstActivation)
            for b in self.main_func.blocks
            for i in b.instructions
        )
        if not has_activation:
            return
        pinned = {AF.Ln, AF.Exp}
        tables = [
            (name, funcs if name == NLE else (funcs - pinned))
            for name, funcs in get_activation_tables(self.m.arch).items()
        ]
        _bass_rust.insert_act_table_loads(self, tables)


def build_program(ln_s: float) -> bass.Bass:
    os.environ.pop("BASS_ACT_ROOT_JSON_PATH", None)
    s_val = float(math.exp(ln_s))
    nc = _BaccPinned(None)
    # host-im2col'd images: [b, k%128, k//128, patch] fp8
    imgs = nc.declare_dram_parameter("imgs", [BPC, 128, NKC, NPAT], FP8, isOutput=False)
    hha = nc.declare_dram_parameter("hha", [BPC, 128, NKC, NPAT], FP8, isOutput=False)
    w1t = nc.declare_dram_parameter("w1t", [128, NKC, D], FP8, isOutput=False)
    w2t = nc.declare_dram_parameter("w2t", [128, NKC, D], FP8, isOutput=False)
    b1 = nc.declare_dram_parameter("b1", [D], F32, isOutput=False)
    b2 = nc.declare_dram_parameter("b2", [D], F32, isOutput=False)
    # row sums of E per (sample, chunk): host does sum(log(.))
    out_rs = nc.declare_dram_parameter("out_rs", [128, BPC * NCHUNK], F32, isOutput=True)
    # rows {0,32,64,96} x [BPC*512]: colsum half0, colsum half1, diag half0, diag half1
    out_rows = nc.declare_dram_parameter("out_rows", [4, BPC * NH], F32, isOutput=True)
    dbg = os.environ.get("BASS_DEBUG_DUMP", "")
    out_dbg = (
        nc.declare_dram_parameter("out_dbg", [128, NPAT], F32, isOutput=True)
        if dbg
        else None
    )

    srcs = (imgs, hha)

    with tile.TileContext(nc) as tc, ExitStack() as ctx:
        p_one = ctx.enter_context(tc.tile_pool(name="singles", bufs=1))
        p_img = ctx.enter_context(tc.tile_pool(name="img", bufs=4))
        p_ym = ctx.enter_context(tc.tile_pool(name="ym", bufs=3))
        p_sq = ctx.enter_context(tc.tile_pool(name="sq", bufs=3))
        p_hat = ctx.enter_context(tc.tile_pool(name="hat", bufs=4))
        p_inv = ctx.enter_context(tc.tile_pool(name="inv", bufs=2))
        p_E = ctx.enter_context(tc.tile_pool(name="E", bufs=3))
        p_t = ctx.enter_context(tc.tile_pool(name="t", bufs=2))
        # PSUM: conv/bc 2 banks + logits 2x2 banks + reduce-rows 2 banks = 8
        pp_cv = ctx.enter_context(tc.tile_pool(name="ppcv", bufs=1, space="PSUM"))
        pp_L = ctx.enter_context(tc.tile_pool(name="ppL", bufs=2, space="PSUM"))
        pp_R = ctx.enter_context(tc.tile_pool(name="ppR", bufs=2, space="PSUM"))

        # constants / weights
        ones_k = p_one.tile([128, 1], BF16)
        nc.vector.memset(ones_k, 1.0)
        bc_st = p_one.tile([128, 128], BF16)  # K=1 broadcast stationaries
        nc.vector.memset(bc_st, 1.0)
        nc.vector.memset(bc_st[0:1, :], s_val)  # a-side rows carry exp(logit_scale)
        nc.vector.memset(bc_st[32:33, :], s_val)
        wts = []
        biases = []
        for wsrc, bsrc in ((w1t, b1), (w2t, b2)):
            wt = p_one.tile([128, NKC, D], FP8, tag=f"wt_{wsrc.name}")
            nc.sync.dma_start(out=wt, in_=wsrc[:])
            wts.append(wt)
            bt = p_one.tile([128, 1], F32, tag=f"bias_{bsrc.name}")
            nc.sync.dma_start(out=bt, in_=bsrc[:].rearrange("(d one) -> d one", one=1))
            biases.append(bt)
        OUT_RS = p_one.tile([128, BPC * NCHUNK], F32)
        OUT2 = p_one.tile([128, BPC * NH], F32)

        # warmup matmuls absorb the weight-DMA waits into PE program order
        R_init = pp_R.tile([128, NH], F32, tag="R", name="R_init")
        for m in range(2):
            nc.tensor.matmul(
                R_init[:, m : m + 1], wts[m][:, 0, :], wts[m][:, 0, 0:1],
                start=True, stop=True,
            )

        st = {}  # per-sample state

        def absorber(out_tile, col, rhs, name):
            """Tiny matmul so the next real matmul keeps a single sync wait.
            Writes [0:1, col] of a region later cleared by a start=True mm."""
            nc.tensor.matmul(
                out_tile[0:1, col : col + 1], ones_k, rhs, start=True, stop=True,
            )

        def conv_m(b, m):
            """conv MMs + PSUM drain (bias, square) for sample b, modality m."""
            img = st[b]["img"][m]
            conv = pp_cv.tile([128, NPAT], F32, tag="cv", name=f"conv_{b}_{m}")
            if m == 0:
                if b > 0:
                    # conv slot WAR vs DVE h_hat-mul(b-1) (bc borrowed it)
                    absorber(conv, 0, st[b - 1]["h_hat"][:, 0:1], f"abA_{b}")
            else:
                # conv slot WAR vs DVE bias_a/sq_a(b)
                absorber(conv, 0, st[b]["ys"][0]["sq"][:, 0:1], f"abB_{b}")
            if USE_DR:
                for t in range(2):
                    for kp in range(NKC // 2):
                        nc.tensor.matmul(
                            conv[:, t * NH : (t + 1) * NH],
                            wts[m][:, 2 * kp : 2 * kp + 2, :],
                            img[:, 2 * kp : 2 * kp + 2, t * NH : (t + 1) * NH],
                            start=(kp == 0), stop=(kp == NKC // 2 - 1),
                            perf_mode=DR,
                        )
            else:
                for t in range(2):
                    for kc in range(NKC):
                        nc.tensor.matmul(
                            conv[:, t * NH : (t + 1) * NH],
                            wts[m][:, kc, :],
                            img[:, kc, t * NH : (t + 1) * NH],
                            start=(kc == 0), stop=(kc == NKC - 1),
                        )
            ym = p_ym.tile([128, NPAT], BF16, tag="ym", name=f"ym_{b}_{m}")
            nc.vector.tensor_scalar_add(ym, conv, biases[m])
            sq = p_sq.tile([128, NPAT], BF16, tag="sq", name=f"sq_{b}_{m}")
            nc.vector.tensor_mul(sq, ym, ym)
            st[b].setdefault("ys", {})[m] = {"ym": ym, "sq": sq}
            if b == 0 and m == 0 and dbg == "ym":
                dbg_dump(ym)

        def norms(b):
            """norm2 rows -> Ln -> Exp(-.5) inverse norms for sample b."""
            ys = st[b]["ys"]
            R1 = pp_R.tile([128, NH], F32, tag="R", name=f"R1_{b}")
            if b > 0:
                # R slot WAR vs ACT Ln(b-1); invs(b-1) is ACT-produced
                absorber(R1, 0, st[b - 1]["invs"][:, 0:1], f"abE_{b}")
            for m in range(2):
                for t in range(2):
                    q = 32 * (2 * m + t)
                    nc.tensor.matmul(
                        R1[q : q + 1, :], ones_k,
                        ys[m]["sq"][:, t * NH : (t + 1) * NH],
                        start=True, stop=True, tile_position=(0, q),
                    )
            u = p_inv.tile([128, NH], F32, tag="u", name=f"u_{b}")
            nc.scalar.activation(out=u[0:97, :], in_=R1[0:97, :], func=AF.Ln)
            invs = p_inv.tile([128, NH], BF16, tag="invs", name=f"invs_{b}")
            nc.scalar.activation(
                out=invs[0:97, :], in_=u[0:97, :], func=AF.Exp, scale=-0.5
            )
            st[b]["invs"] = invs

        def bc_m(b, m):
            """broadcast inv rows into the conv slot; hat = ym * bc."""
            ys = st[b]["ys"]
            invs = st[b]["invs"]
            bc = pp_cv.tile([128, NPAT], F32, tag="cv", name=f"bc_{b}_{m}")
            if m == 0:
                # conv slot WAR vs DVE bias_h(b): ym_h is bias_h's output
                absorber(bc, 0, ys[1]["ym"][:, 0:1], f"abC_{b}")
            for t in range(2):
                q = 32 * (2 * m + t)
                nc.tensor.matmul(
                    bc[:, t * NH : (t + 1) * NH],
                    bc_st[q : q + 1, :], invs[q : q + 1, :],
                    start=True, stop=True, tile_position=(q, 0),
                )
            hat = p_hat.tile([128, NPAT], BF16, tag="hat", name=f"hat_{b}_{m}")
            nc.vector.tensor_mul(hat, ys[m]["ym"], bc)
            st[b]["a_hat" if m == 0 else "h_hat"] = hat

        def iteration(i):
            """consume(i) chunk loop with produce stages of i+1/i+2 woven in."""
            a_hat, h_hat = st[i]["a_hat"], st[i]["h_hat"]
            R2 = pp_R.tile([128, NH], F32, tag="R", name=f"R2_{i}")
            absorber(R2, 0, h_hat[:, 0:1], f"abD_{i}")
            for k in range(NCHUNK):
                L = pp_L.tile([128, NPAT], F32, tag="L", name=f"L_{i}_{k}")
                for t in range(2):
                    nc.tensor.matmul(
                        L[:, t * NH : (t + 1) * NH],
                        a_hat[:, 128 * k : 128 * (k + 1)],
                        h_hat[:, t * NH : (t + 1) * NH],
                        start=True, stop=True,
                    )
                E = p_E.tile([128, NPAT], BF16, tag="E", name=f"E_{i}_{k}")
                nc.scalar.activation(
                    out=E, in_=L, func=AF.Exp,
                    accum_out=OUT_RS[:, NCHUNK * i + k : NCHUNK * i + k + 1],
                )
                for t in range(2):
                    nc.tensor.matmul(
                        R2[32 * t : 32 * t + 1, :], ones_k,
                        E[:, t * NH : (t + 1) * NH],
                        start=(k == 0), stop=(k == NCHUNK - 1),
                        tile_position=(0, 32 * t),
                    )
                if k == 1 and i + 1 < BPC:
                    norms(i + 1)
                elif k == 3 and i + 1 < BPC:
                    bc_m(i + 1, 0)
                elif k == 4 and i + 1 < BPC:
                    bc_m(i + 1, 1)
                elif k == 5 and i + 2 < BPC:
                    conv_m(i + 2, 0)
                elif k == 6 and i + 2 < BPC:
                    conv_m(i + 2, 1)
            t_ = p_t.tile([128, NPAT], BF16, tag="t", name=f"t_{i}")
            nc.vector.tensor_mul(t_, a_hat, h_hat)
            for t in range(2):
                nc.tensor.matmul(
                    R2[64 + 32 * t : 64 + 32 * t + 1, :], ones_k,
                    t_[:, t * NH : (t + 1) * NH],
                    start=True, stop=True, tile_position=(0, 64 + 32 * t),
                )
            nc.vector.tensor_copy(OUT2[0:97, i * NH : (i + 1) * NH], R2[0:97, :])
            if i + 3 < BPC:
                dma_imgs(i + 3)

        def dma_imgs(b):
            img_a = p_img.tile([128, NKC, NPAT], FP8, tag="img", name=f"ia_{b}")
            nc.sync.dma_start(out=img_a, in_=srcs[0][b])
            img_h = p_img.tile([128, NKC, NPAT], FP8, tag="img", name=f"ih_{b}")
            nc.sync.dma_start(out=img_h, in_=srcs[1][b])
            st[b] = {"img": (img_a, img_h)}

        DBG = p_one.tile([128, NPAT], F32, name="DBG") if dbg else None

        def dbg_dump(src):
            if DBG is not None:
                nc.vector.tensor_copy(DBG, src)

        # prologue: samples 0 and 1 produced serially, then steady iterations
        dma_imgs(0)
        dma_imgs(1)
        dma_imgs(2)
        conv_m(0, 0)
        conv_m(0, 1)
        norms(0)
        bc_m(0, 0)
        bc_m(0, 1)
        conv_m(1, 0)
        conv_m(1, 1)
        for i in range(BPC):
            iteration(i)
        nc.sync.dma_start(out=out_rs[:], in_=OUT_RS)
        nc.sync.dma_start(out=out_rows[:], in_=OUT2[0:128:32, :])
        if DBG is not None:
            nc.sync.dma_start(out=out_dbg[:], in_=DBG)

    nc.compile()
    return nc


_PROGRAM_CACHE: dict = {}


def _get_program(ln_s: float) -> bass.Bass:
    key = round(float(ln_s), 9)
    if key not in _PROGRAM_CACHE:
        _PROGRAM_CACHE[key] = build_program(float(ln_s))
    return _PROGRAM_CACHE[key]


def make_in_maps(imgs, hha, w1, b1, w2, b2):
    """Shard full inputs into per-core input maps (host-side, cheap)."""
    import ml_dtypes

    fp8 = ml_dtypes.float8_e4m3

    def prep_w(w):
        # [D,C,P,P] -> [(c,di,dj)=1024, D] -> [k%128, k//128, D]
        wf = np.transpose(np.asarray(w), (1, 2, 3, 0)).reshape(C * NOFF, D)
        return np.ascontiguousarray(
            wf.reshape(NKC, 128, D).transpose(1, 0, 2)
        ).astype(fp8)

    def prep_x(x):
        # stride==kernel -> im2col is a permutation:
        # [B,C,H,W] -> [B, (c,di,dj)=1024, (i,j)=1024] -> [B,128,NKC,NPAT]
        B = x.shape[0]
        xp = np.asarray(x).reshape(B, C, H // P, P, W // P, P)
        xp = xp.transpose(0, 1, 3, 5, 2, 4).reshape(B, C * NOFF, NPAT)
        return np.ascontiguousarray(
            xp.reshape(B, NKC, 128, NPAT).transpose(0, 2, 1, 3)
        ).astype(fp8)

    w1t = prep_w(w1)
    w2t = prep_w(w2)
    imgs = prep_x(imgs)
    hha = prep_x(hha)
    b1 = np.ascontiguousarray(np.asarray(b1), dtype=np.float32)
    b2 = np.ascontiguousarray(np.asarray(b2), dtype=np.float32)
    maps = []
    for i in range(N_CORES):
        maps.append(
            {
                "imgs": np.ascontiguousarray(imgs[i * BPC : (i + 1) * BPC]),
                "hha": np.ascontiguousarray(hha[i * BPC : (i + 1) * BPC]),
                "w1t": w1t,
                "w2t": w2t,
                "b1": b1,
                "b2": b2,
            }
        )
    return maps


def combine_outputs(outs) -> np.float32:
    """Reduce per-core {out_rs, out_rows} partials to the scalar loss."""
    tot = np.float64(0.0)
    for o in outs:
        rs = np.asarray(o["out_rs"], dtype=np.float64)  # [128, BPC*8]
        rows = np.asarray(o["out_rows"], dtype=np.float64)  # [4, BPC*512]
        lse_row = np.log(rs).sum()
        colsum = np.concatenate([rows[0], rows[1]])  # [BPC*512]*2 -> all cols
        lse_col = np.log(colsum).sum()
        diag = rows[2].sum() + rows[3].sum()
        tot += 0.5 * (lse_row + lse_col) - diag
    return np.float32(tot / (B_FULL * NPAT))


def run_spmd(imgs, hha, w1, b1, w2, b2, logit_scale, **kwargs):
    """Run on the 8 cores; returns (loss, BassKernelResults)."""
    ln_s = float(np.asarray(logit_scale))
    nc = _get_program(ln_s)
    in_maps = make_in_maps(imgs, hha, w1, b1, w2, b2)
    res = run_bass_kernel_spmd(nc, in_maps, list(range(N_CORES)), **kwargs)
    return combine_outputs(res.results), res


def kernel(imgs, hha, w1, b1, w2, b2, logit_scale):
    loss, _ = run_spmd(imgs, hha, w1, b1, w2, b2, logit_scale)
    return loss


if __name__ == "__main__":
    # smoke test against a tiny numpy reference of the math
    rng = np.random.default_rng(0)
    imgs = rng.standard_normal((B_FULL, C, H, W), dtype=np.float32)
    hha = rng.standard_normal((B_FULL, C, H, W), dtype=np.float32)
    w1 = rng.standard_normal((D, C, P, P), dtype=np.float32) * 0.03
    w2 = rng.standard_normal((D, C, P, P), dtype=np.float32) * 0.03
    b1 = np.zeros(D, np.float32)
    b2 = np.zeros(D, np.float32)
    ls = np.float32(np.log(1.0 / 0.07))
    print(kernel(imgs, hha, w1, b1, w2, b2, ls))
